# revision 1
# baseline (speedup 1.0000x reference)
"""BSRoformer attention kernel for 8 TRN2 NeuronCores.

Sharding: 8 cores = 4 batch elements x 2 tensor-parallel halves.
Core c handles batch b = c//2, TP half t = c%2 (q heads 8t..8t+8,
kv heads 2t..2t+2, Wo columns 512t..512t+512). Host sums the two
row-parallel O-projection partials per batch element.

Device pipeline (everything in "transposed" token-last layout):
  xT [1024,2048] --fp32r matmul--> qT/kT (RoPE on DVE) and V (natural)
  scores S^T[k,q] per head via K=64 matmuls (head pairs packed into
  PE row groups), exp on ACT (scale=1/8 folded in), P^T in bf16,
  P^T @ [V|1] gives out^T plus softmax denominator row, normalize via
  reciprocal_approx + partition-broadcast DMA, bf16 O-projection.
"""

import os
import sys
from contextlib import ExitStack

import numpy as np

try:
    import concourse.bass as bass  # noqa: F401
except Exception:  # pragma: no cover
    sys.path.insert(0, "/opt/trn_rl_repo")
    import concourse.bass as bass  # noqa: F401

import ml_dtypes
import concourse.mybir as mybir
import concourse.tile as tile
from concourse import bacc
from concourse.bass_utils import run_bass_kernel_spmd

F32 = mybir.dt.float32
F32R = mybir.dt.float32r
BF16 = mybir.dt.bfloat16

B, S, HID = 4, 2048, 1024
NH, NKV, HD = 16, 4, 64
SCALE = HD ** -0.5
NHL, NKVL = 8, 2          # per-core q heads / kv heads (TP half)
QD, KD = NHL * HD, NKVL * HD   # 512, 128
VD = KD + NKVL            # 130: [v_g0(64) | ones | v_g1(64) | ones]
NPAIR = NHL // 2          # 4 head pairs per core
NKT = S // 128            # 16 key tiles
KTG = 2                   # key tiles per psum scores group
NQC = S // 512            # 4 query chunks

LAST_RESULTS = None


def _f32r(ap):
    return ap.bitcast(F32R)


DEBUG = False


def build_bass():
    nc = bacc.Bacc()
    inp = {}
    for name, shape, dt in [
        ("xT", [HID, S], BF16),
        ("wqT", [HID + 1, QD], BF16),
        ("wkT", [HID + 1, KD], BF16),
        ("wvT", [HID + 1, VD], BF16),
        ("woT", [QD, HID], BF16),
        ("oneh", [8, 512], F32),
        ("cosT2", [128, S], F32),
        ("ssinT2", [128, S], F32),
    ]:
        inp[name] = nc.declare_dram_parameter(name, shape, dt, isOutput=False).ap()
    out = nc.declare_dram_parameter("out", [S, HID], F32, isOutput=True).ap()
    dbg = {}
    if DEBUG:
        for name, shape, dt in [
            ("dbg_qt0", [128, S], BF16), ("dbg_ktd0", [128, S], BF16),
            ("dbg_v0", [128, VD], BF16), ("dbg_pt", [128, NKT, 512], BF16),
            ("dbg_pv", [65, 512], F32), ("dbg_bc", [64, 512], F32),
            ("dbg_at", [128, 512], BF16),
        ]:
            dbg[name] = nc.declare_dram_parameter(name, shape, dt, isOutput=True).ap()

    with ExitStack() as ctx:
        tc = ctx.enter_context(tile.TileContext(nc))
        # ---------------- persistent pools ----------------
        consts = ctx.enter_context(tc.tile_pool(name="consts", bufs=1))
        qtp = ctx.enter_context(tc.tile_pool(name="qtp", bufs=1))
        ktp = ctx.enter_context(tc.tile_pool(name="ktp", bufs=1))
        vp = ctx.enter_context(tc.tile_pool(name="vp", bufs=1))
        # PSUM: pj(2 banks, shared with O-proj) + sc(2x2) + pv(2x1) = 8 banks
        pj = ctx.enter_context(tc.tile_pool(name="pj", bufs=2, space="PSUM"))
        # projection-phase pools (freed before attention SBUF pools allocate)
        prj = ctx.enter_context(tc.tile_pool(name="prj", bufs=1))
        tsp = ctx.enter_context(tc.tile_pool(name="tsp", bufs=4))

        # ---------------- constants / weights in SBUF ----------------
        cos_sb = prj.tile([128, S], F32, tag="cos", name="cos")
        sin_sb = prj.tile([128, S], F32, tag="sin", name="sin")
        nc.sync.dma_start(out=cos_sb, in_=inp["cosT2"])
        nc.sync.dma_start(out=sin_sb, in_=inp["ssinT2"])
        ones_f = consts.tile([1, S], BF16, tag="ones_f", name="ones_f")
        nc.vector.memset(ones_f, 1.0)
        oneh = consts.tile([8, 512], F32, tag="oneh", name="oneh")
        nc.gpsimd.dma_start(out=oneh, in_=inp["oneh"])

        wq = [prj.tile([128, QD], BF16, tag=f"wq{c}", name=f"wq{c}") for c in range(8)]
        wk = [prj.tile([128, KD], BF16, tag=f"wk{c}", name=f"wk{c}") for c in range(8)]
        wv = [prj.tile([128, VD], BF16, tag=f"wv{c}", name=f"wv{c}") for c in range(8)]
        for c in range(8):
            nc.sync.dma_start(out=wq[c], in_=inp["wqT"][c * 128:(c + 1) * 128, :])
            nc.sync.dma_start(out=wk[c], in_=inp["wkT"][c * 128:(c + 1) * 128, :])
            nc.sync.dma_start(out=wv[c], in_=inp["wvT"][c * 128:(c + 1) * 128, :])
        wq_b = prj.tile([1, QD], BF16, tag="wq_b", name="wq_b")
        wk_b = prj.tile([1, KD], BF16, tag="wk_b", name="wk_b")
        wv_b = prj.tile([1, VD], BF16, tag="wv_b", name="wv_b")
        nc.sync.dma_start(out=wq_b, in_=inp["wqT"][HID:HID + 1, :])
        nc.sync.dma_start(out=wk_b, in_=inp["wkT"][HID:HID + 1, :])
        nc.sync.dma_start(out=wv_b, in_=inp["wvT"][HID:HID + 1, :])
        wo = [consts.tile([128, HID], BF16, tag=f"wo{c}", name=f"wo{c}") for c in range(4)]
        for c in range(4):
            nc.sync.dma_start(out=wo[c], in_=inp["woT"][c * 128:(c + 1) * 128, :])
        xt = [prj.tile([128, S], BF16, tag=f"xt{c}", name=f"xt{c}") for c in range(8)]
        for c in range(8):
            nc.sync.dma_start(out=xt[c], in_=inp["xT"][c * 128:(c + 1) * 128, :])

        # ---------------- projections + RoPE ----------------
        def rope_evac(ps, dst, qs):
            """dst[:, qs] = rope(ps) using cos/sgn-sin tiles (per 64-block)."""
            ts_ = tsp.tile([128, 512], F32, tag="ts", name="ts")
            nc.vector.tensor_mul(ts_[0:32, :], ps[32:64, :], sin_sb[0:32, qs])
            nc.vector.tensor_mul(ts_[32:64, :], ps[0:32, :], sin_sb[32:64, qs])
            nc.vector.tensor_mul(ts_[64:96, :], ps[96:128, :], sin_sb[64:96, qs])
            nc.vector.tensor_mul(ts_[96:128, :], ps[64:96, :], sin_sb[96:128, qs])
            nc.vector.tensor_mul(dst[:, qs], ps, cos_sb[:, qs])
            nc.vector.tensor_add(dst[:, qs], dst[:, qs], ts_)

        # K projection -> kt_raw, then duplicate per kv head
        kt_raw = prj.tile([128, S], BF16, tag="kt_raw", name="kt_raw")
        for q in range(NQC):
            qs = slice(q * 512, (q + 1) * 512)
            ps = pj.tile([128, 512], F32, tag="pj", name="pj")
            for c in range(8):
                nc.tensor.matmul(ps, (wk[c]), (xt[c][:, qs]),
                                 start=(c == 0), stop=False)
            nc.tensor.matmul(ps, (wk_b), (ones_f[0:1, qs]),
                             start=False, stop=True)
            rope_evac(ps, kt_raw, qs)
        ktd = [ktp.tile([128, S], BF16, tag=f"ktd{g}", name=f"ktd{g}") for g in range(2)]
        for g in range(2):
            src = kt_raw[g * 64:(g + 1) * 64, :]
            nc.sync.dma_start(out=ktd[g][0:64, :], in_=src)
            nc.sync.dma_start(out=ktd[g][64:128, :], in_=src)

        # V projection (natural layout, with ones columns)
        vt = [vp.tile([128, VD], BF16, tag=f"v{i}", name=f"v{i}") for i in range(NKT)]
        for i in range(NKT):
            tsl = slice(i * 128, (i + 1) * 128)
            ps = pj.tile([128, 512], F32, tag="pj", name="pj")
            psv = ps[:, 0:VD]
            for c in range(8):
                nc.tensor.matmul(psv, (xt[c][:, tsl]), (wv[c]),
                                 start=(c == 0), stop=False)
            nc.tensor.matmul(psv, (ones_f[0:1, tsl]), (wv_b),
                             start=False, stop=True)
            nc.vector.tensor_copy(vt[i], psv)

        # Q projection tiles (2 heads each); chunks produced inside qc loop
        qt = [qtp.tile([128, S], BF16, tag=f"qt{i}", name=f"qt{i}") for i in range(4)]

        if DEBUG:
            nc.sync.dma_start(out=dbg["dbg_qt0"], in_=qt[0])
            nc.sync.dma_start(out=dbg["dbg_ktd0"], in_=ktd[0])
            nc.sync.dma_start(out=dbg["dbg_v0"], in_=vt[0])
        # ---------------- attention pools ----------------
        ptp = ctx.enter_context(tc.tile_pool(name="ptp", bufs=6))
        atp = ctx.enter_context(tc.tile_pool(name="atp", bufs=8))
        rcp = ctx.enter_context(tc.tile_pool(name="rcp", bufs=2))
        unp = ctx.enter_context(tc.tile_pool(name="unp", bufs=8))
        dnp = ctx.enter_context(tc.tile_pool(name="dnp", bufs=2))
        bcp = ctx.enter_context(tc.tile_pool(name="bcp", bufs=4))
        osp = ctx.enter_context(tc.tile_pool(name="osp", bufs=4))
        scp = ctx.enter_context(tc.tile_pool(name="scp", bufs=2, space="PSUM"))
        pvp = ctx.enter_context(tc.tile_pool(name="pvp", bufs=2, space="PSUM"))

        # ---------------- attention + O-projection ----------------
        for q in range(NQC):
            qs = slice(q * 512, (q + 1) * 512)
            for i in range(4):
                msl = slice(i * 128, (i + 1) * 128)
                ps = pj.tile([128, 512], F32, tag="pj", name="pj")
                for c in range(8):
                    nc.tensor.matmul(ps, (wq[c][:, msl]), (xt[c][:, qs]),
                                     start=(c == 0), stop=False)
                nc.tensor.matmul(ps, (wq_b[:, msl]), (ones_f[0:1, qs]),
                                 start=False, stop=True)
                rope_evac(ps, qt[i], qs)
            at = [atp.tile([128, 512], BF16, tag="at", name="at") for _ in range(NPAIR)]
            dn = dnp.tile([8, 512], F32, tag="dn", name="dn")
            uns = []
            for p in range(NPAIR):
                g = p // 2
                pv_ab = [pvp.tile([65, 512], F32, tag="pv", name="pv") for _ in range(2)]
                for kg in range(NKT // KTG):
                    sc_ab = [scp.tile([128, KTG * 512], F32, tag="sc", name="sc")
                             for _ in range(2)]
                    for ab in range(2):
                        qrow = slice(ab * 64, (ab + 1) * 64)
                        for j in range(KTG):
                            kt_i = kg * KTG + j
                            ksl = slice(kt_i * 128, (kt_i + 1) * 128)
                            nc.tensor.matmul(
                                sc_ab[ab][:, j * 512:(j + 1) * 512],
                                (ktd[g][qrow, ksl]),
                                (qt[p][qrow, qs]),
                                start=True, stop=True)
                        pt_g = ptp.tile([128, KTG, 512], BF16, tag="pt", name="pt")
                        nc.scalar.activation(
                            out=pt_g,
                            in_=sc_ab[ab],
                            func=mybir.ActivationFunctionType.Exp,
                            scale=SCALE)
                        for j in range(KTG):
                            kt_i = kg * KTG + j
                            nc.tensor.matmul(
                                pv_ab[ab],
                                vt[kt_i][:, g * 65:(g + 1) * 65],
                                pt_g[:, j, :],
                                start=(kt_i == 0), stop=(kt_i == NKT - 1))
                un = unp.tile([128, 512], F32, tag="un", name="un")
                nc.vector.tensor_copy(un[0:64, :], pv_ab[0][0:64, :])
                nc.vector.tensor_copy(un[64:128, :], pv_ab[1][0:64, :])
                for ab in range(2):
                    rt = rcp.tile([1, 512], F32, tag="rt", name="rt")
                    nc.vector.tensor_copy(rt, pv_ab[ab][64:65, :])
                    nc.gpsimd.dma_start(out=dn[2 * p + ab:2 * p + ab + 1, :], in_=rt)
                uns.append(un)
            rc8 = rcp.tile([8, 512], F32, tag="rc", name="rc")
            nc.vector.reciprocal(rc8, dn)
            for p in range(NPAIR):
                bc_ps = pj.tile([128, 512], F32, tag="pj", name="pj")
                nc.tensor.matmul(bc_ps,
                                 oneh[:, p * 128:(p + 1) * 128], rc8,
                                 start=True, stop=True)
                nc.vector.tensor_mul(at[p], uns[p], bc_ps)
            if DEBUG and q == 0:
                nc.sync.dma_start(out=dbg["dbg_at"], in_=at[0])
            # O-projection for this query chunk
            for t_ in range(4):
                tsl = slice(t_ * 128, (t_ + 1) * 128)
                rows = slice(q * 512 + t_ * 128, q * 512 + (t_ + 1) * 128)
                for nh in range(2):
                    nsl = slice(nh * 512, (nh + 1) * 512)
                    po = pj.tile([128, 512], F32, tag="pj", name="pj")
                    for p in range(NPAIR):
                        nc.tensor.matmul(po, at[p][:, tsl], wo[p][:, nsl],
                                         start=(p == 0), stop=(p == NPAIR - 1))
                    os_ = osp.tile([128, 512], F32, tag="os", name="os")
                    nc.vector.tensor_copy(os_, po)
                    nc.sync.dma_start(out=out[rows, nsl], in_=os_)
    if hasattr(nc, "compile"):
        nc.compile()
    return nc


_NC_CACHE = None


def _get_nc():
    global _NC_CACHE
    if _NC_CACHE is None:
        _NC_CACHE = build_bass()
    return _NC_CACHE


def _prep_core_inputs(hs, cos, sin, Wq, bq, Wk, bk, Wv, bv, Wo, bo):
    """Build the 8 per-core input dicts (host-side shard + transpose)."""
    cosT = np.ascontiguousarray(cos.reshape(S, HD).T)          # [64, S]
    sinT = np.ascontiguousarray(sin.reshape(S, HD).T)
    cosT2 = np.tile(cosT, (2, 1)).astype(np.float32)            # [128, S]
    oneh = np.zeros((8, 512), np.float32)
    for h in range(8):
        oneh[h, h * 64:(h + 1) * 64] = 1.0
    ssinT = sinT.copy()
    ssinT[0:HD // 2] = -ssinT[0:HD // 2]
    ssinT2 = np.tile(ssinT, (2, 1)).astype(np.float32)

    in_maps = []
    for c in range(8):
        b, t = c // 2, c % 2
        xT = np.ascontiguousarray(hs[b].T).astype(ml_dtypes.bfloat16)  # [1024, S]

        wqT = np.empty((HID + 1, QD), ml_dtypes.bfloat16)
        wqT[:HID] = Wq[t * QD:(t + 1) * QD].T
        wqT[HID] = bq[t * QD:(t + 1) * QD]

        wkT = np.empty((HID + 1, KD), ml_dtypes.bfloat16)
        wkT[:HID] = Wk[t * KD:(t + 1) * KD].T
        wkT[HID] = bk[t * KD:(t + 1) * KD]

        wvT = np.zeros((HID + 1, VD), ml_dtypes.bfloat16)
        for g in range(NKVL):
            src = Wv[t * KD + g * HD: t * KD + (g + 1) * HD].T   # [1024, 64]
            wvT[:HID, g * 65:g * 65 + HD] = src
            wvT[HID, g * 65:g * 65 + HD] = bv[t * KD + g * HD: t * KD + (g + 1) * HD]
            wvT[HID, g * 65 + HD] = 1.0

        woT = np.ascontiguousarray(
            Wo[:, t * QD:(t + 1) * QD].T).astype(ml_dtypes.bfloat16)  # [512, 1024]
        in_maps.append(dict(
            xT=xT, wqT=wqT, wkT=wkT, wvT=wvT, woT=woT, oneh=oneh,
            cosT2=cosT2, ssinT2=ssinT2,
        ))
    return in_maps


def kernel(hidden_states, cos, sin, Wq, bq, Wk, bk, Wv, bv, Wo, bo,
           _trace=False, _trace_kwargs=None):
    global LAST_RESULTS
    args = [np.asarray(a, dtype=np.float32) for a in
            (hidden_states, cos, sin, Wq, bq, Wk, bk, Wv, bv, Wo, bo)]
    in_maps = _prep_core_inputs(*args)
    nc = _get_nc()
    kw = dict(_trace_kwargs or {})
    res = run_bass_kernel_spmd(nc, in_maps, core_ids=list(range(8)),
                               trace=_trace, **kw)
    LAST_RESULTS = res
    outs = [r["out"] for r in res.results]
    full = np.empty((B, S, HID), np.float32)
    bo = args[10]
    for b in range(B):
        full[b] = outs[2 * b] + outs[2 * b + 1] + bo[None, :]
    return full

